# revision 1
# baseline (speedup 1.0000x reference)
"""AdaFreqFilter (channel_wise) distributed Bass kernel for 8 TRN2 cores.

out = H - K * (H - rsqrt(deg_in) * segsum_dst(H[src] * rsqrt(deg_out)[src]))
    = (1-K)*H + rsqrt(deg_in) * segsum_dst((H * K * rsqrt(deg_out))[src])

Strategy (standard distributed SpMM row partitioning):
  - Output rows (dst nodes) are partitioned across 8 cores; the host
    routes each edge to the core owning its dst (pure index shuffling).
  - hk = H * rsqrt(deg_out)[node] * K[feat] is materialized in device
    DRAM (bf16) with one fused scalar_tensor_tensor per 128-row tile.
  - Within a core, edges are grouped by 128-node dst block and padded to
    128-edge chunks.  Each chunk is aggregated with one TensorE matmul:
    P^T @ M where P is a one-hot (edge -> local dst bin) built on-chip
    from iota + is_equal, and M is the chunk's gathered hk rows.
    Chunks of a block accumulate natively in PSUM.
  - Edge rows are fetched with dma_gather (indices limited to int16, so
    each block's edges are split into pass A (src < 32768) and pass B
    (src >= 32768, gathered from an offset base)).
  - Degrees come from CSR rowptrs (host-sorted edge lists); the device
    computes deg = ptr_hi - ptr_lo, clamps, and takes rsqrt.
  - Epilogue per block is a single op: out = psum * rsqrt(deg_in) + hepiK
    where hepiK = (1-K)*H was prepared during the build phase.
"""

from contextlib import ExitStack

import ml_dtypes
import numpy as np

from concourse import bacc, bass, mybir
from concourse.bass_utils import run_bass_kernel_spmd

F32 = mybir.dt.float32
BF16 = mybir.dt.bfloat16
I16 = mybir.dt.int16
OP = mybir.AluOpType
ACT = mybir.ActivationFunctionType

DEF_CFG = dict(
    N=50000,      # nodes
    D=128,        # features
    CORES=8,
    SPLIT=32768,  # int16 gather index limit
    GCH=8,       # chunks per gather call (swdge ring: 1024 descriptors)
    NSLOT=8,      # gather buffer slots per stream
    HBT=16,       # H tiles per phase-1 DMA group
    PG=8,         # chunks per P-build op
    NPSUM=6,      # rotating PSUM tiles
    PAD_DLOC=200.0,  # sentinel local-dst for padding edges (no one-hot match)
)


# --------------------------------------------------------------------------
# Host-side plan: shard + sort + pad the edge lists (index manipulation only)
# --------------------------------------------------------------------------

def make_plan(src, dst, cfg=None):
    cfg = dict(DEF_CFG, **(cfg or {}))
    N, CORES, SPLIT, GCH = cfg["N"], cfg["CORES"], cfg["SPLIT"], cfg["GCH"]
    PG = cfg["PG"]
    src = np.asarray(src).astype(np.int64)
    dst = np.asarray(dst).astype(np.int64)
    E = src.shape[0]

    NPC = N // CORES                      # nodes per core
    assert NPC * CORES == N
    NB = (NPC + 127) // 128               # dst blocks per core
    NPAD = NB * 128
    NT = (N + 127) // 128                 # hk tiles (global nodes)

    # global src-sorted rowptr (for deg_out), sliced per core below
    NBS = (NPC + 127) // 128              # build tiles per core slice
    ssrc = np.sort(src)
    rowptrS = np.searchsorted(ssrc, np.arange(NT * 128 + 1)).astype(np.int64)
    rs_pc = []  # filled after permutations are computed

    # per-core edge routing with block-balancing permutation.
    # Each core's nodes are permuted WITHIN their half (so src-side pass
    # assignment stays fixed) to equalize per-(block, pass) edge counts,
    # which shrinks the ceil-padded chunk schedule.
    HALF = NPC // 2
    core_of = dst // NPC
    src_half_q = ((src % NPC) >= HALF).astype(np.int64)
    # per-dst-node in-edge counts split by source half
    cntq = np.zeros((N, 2), dtype=np.int64)
    np.add.at(cntq, (dst, src_half_q), 1)

    orders, pos_ofs = [], []
    OVR = 3  # overflow blocks shared across cores (first OVR per half-range)
    for c in range(CORES):
        w = cntq[c * NPC:(c + 1) * NPC].astype(np.float64)   # [NPC, 2]
        slots = {b: [] for b in range(NB)}
        bsum = np.zeros((NB, 2))
        for half in (1, 0):
            lo = 0 if half == 0 else HALF
            hi = HALF if half == 0 else NPC
            bl, caps, lim = [], [], []
            for b in range(NB):
                b0, b1 = b * 128, min((b + 1) * 128, NPC)
                ov = max(0, min(b1, hi) - max(b0, lo))
                if ov:
                    bl.append(b); caps.append(ov)
                    lim.append(896.0 if len(bl) <= OVR else 768.0)
            bl = np.array(bl); caps = np.array(caps); lim = np.array(lim)
            sA = bsum[bl, 0].copy(); sB = bsum[bl, 1].copy()
            nodes = lo + np.argsort(-(w[lo:hi, 0] + w[lo:hi, 1]), kind="stable")
            for n in nodes:
                score = np.maximum((sA + w[n, 0]) / lim, (sB + w[n, 1]) / lim)
                score[caps == 0] = np.inf
                i = int(np.argmin(score))
                slots[int(bl[i])].append(int(n))
                caps[i] -= 1; sA[i] += w[n, 0]; sB[i] += w[n, 1]
            bsum[bl, 0] = sA; bsum[bl, 1] = sB
        order = np.empty(NPC, dtype=np.int64)
        for b in range(NB):
            b0, b1 = b * 128, min((b + 1) * 128, NPC)
            n0 = sorted(n for n in slots[b] if n < HALF)
            n1 = sorted(n for n in slots[b] if n >= HALF)
            k0 = max(0, min(b1, HALF) - b0)
            assert len(n0) == k0 and len(n1) == (b1 - b0) - k0, (b, len(n0), k0)
            order[b0:b0 + k0] = n0
            order[b0 + k0:b1] = n1
        pos_of = np.empty(NPC, dtype=np.int64)
        pos_of[order] = np.arange(NPC)
        orders.append(order)
        pos_ofs.append(pos_of)

    per_core = []
    counts = np.zeros((CORES, NB, 2), dtype=np.int64)
    core_edges = []
    for c in range(CORES):
        m = core_of == c
        s_c, d_raw = src[m], dst[m] - c * NPC
        d_c = pos_ofs[c][d_raw]                   # permuted dst position
        blk = d_c // 128
        # permuted src position within its core (within-half perm keeps
        # the half membership, so pass == original half)
        s_pos = np.empty_like(s_c)
        for cc in range(CORES):
            mm_ = (s_c // NPC) == cc
            s_pos[mm_] = pos_ofs[cc][s_c[mm_] % NPC]
        pas = (s_pos >= HALF).astype(np.int64)
        gidx = (s_c // NPC) * HALF + np.where(pas == 0, s_pos, s_pos - HALF)
        core_edges.append((gidx, d_c, blk, pas))
        np.add.at(counts[c], (blk, pas), 1)

    # global chunk schedule: K[b][q] = chunks for block b pass q (same all cores)
    K = np.ceil(counts.max(axis=0) / 128.0).astype(np.int64)  # [NB, 2]
    for b in range(NB):
        if K[b].sum() == 0:
            K[b][0] = 1
    nA, nB_ = int(K[:, 0].sum()), int(K[:, 1].sum())
    NCHUNK = nA + nB_
    cumq = [np.concatenate([[0], np.cumsum(K[:, q])]).astype(np.int64) for q in (0, 1)]

    # gather calls per stream: spans of <= GCH chunks
    calls = []  # (q, chunk_start_in_q, nch, callid_in_q)
    for q, nq in ((0, nA), (1, nB_)):
        for k, s0 in enumerate(range(0, nq, GCH)):
            calls.append((q, s0, min(GCH, nq - s0), k))

    # matmul consumption order: block-major, pass A then B within a block
    mm = []
    for b in range(NB):
        nchunks_b = int(K[b].sum())
        j = 0
        for q in (0, 1):
            for i in range(int(K[b][q])):
                posq = int(cumq[q][b]) + i
                mm.append(dict(
                    sid=(posq if q == 0 else nA + posq), q=q, posq=posq, b=b,
                    callq=posq // GCH,
                    first=(j == 0), last=(j == nchunks_b - 1),
                ))
                j += 1
    assert len(mm) == NCHUNK
    last_mm_of_block, last_mm_of_call, first_mm_of_call = {}, {}, {}
    first_mm_of_pg, last_mm_of_pg = {}, {}
    for i, e in enumerate(mm):
        last_mm_of_block[e["b"]] = i
        last_mm_of_call[(e["q"], e["callq"])] = i
        first_mm_of_call.setdefault((e["q"], e["callq"]), i)
        pg = (e["q"], e["posq"] // PG)
        first_mm_of_pg.setdefault(pg, i)
        last_mm_of_pg[pg] = i

    # P-build groups of PG chunks per stream, ordered by first consumption
    pgroups = []
    for q, nq in ((0, nA), (1, nB_)):
        for j in range((nq + PG - 1) // PG):
            pgroups.append((q, j, j * PG, min(PG, nq - j * PG)))
    pgroups.sort(key=lambda g: first_mm_of_pg[(g[0], g[1])])

    # per-core padded index / dloc arrays
    pad_dloc = cfg["PAD_DLOC"]
    for c in range(CORES):
        s_c, d_c, blk, pas = core_edges[c]
        idx = np.zeros(NCHUNK * 128, dtype=np.int16)
        dloc = np.full(NCHUNK * 128, pad_dloc, dtype=np.float32)
        for q in (0, 1):
            base = 0 if q == 0 else nA
            sel = pas == q
            sq, dq, bq = s_c[sel], d_c[sel], blk[sel]
            order = np.argsort(bq, kind="stable")
            sq, dq, bq = sq[order], dq[order], bq[order]
            bstart = np.concatenate([[0], np.cumsum(np.bincount(bq, minlength=NB))])
            for b in range(NB):
                e0, e1 = int(bstart[b]), int(bstart[b + 1])
                n = e1 - e0
                off = (base + int(cumq[q][b])) * 128
                idx[off:off + n] = sq[e0:e1].astype(np.int16)
                dloc[off:off + n] = (dq[e0:e1] - b * 128).astype(np.float32)
        # wrap indices (i -> [i%16, i//16]), replicated over the 8 gpsimd cores
        idxw = np.tile(idx.reshape(NCHUNK * 8, 16).T, (8, 1)).copy()  # [128, NCHUNK*8]
        dlocw = dloc.reshape(NCHUNK, 128).T.copy()    # [128, NCHUNK]
        # deg_in rowptr strips from this core's sorted PERMUTED local dst
        sd = np.sort(core_edges[c][1])
        nodes = np.arange(NPAD)
        rdA = np.searchsorted(sd, nodes).astype(np.float32).reshape(NB, 128).T.copy()
        rdB = np.searchsorted(sd, nodes + 1).astype(np.float32).reshape(NB, 128).T.copy()
        # deg_out strips in permuted order
        ordext = np.concatenate([orders[c] + c * NPC,
                                 np.minimum(c * NPC + np.arange(NPC, NPAD), NT * 128 - 1)])
        lo = rowptrS[ordext.astype(np.int64)].astype(np.float32).reshape(NB, 128).T.copy()
        hi = rowptrS[np.minimum(ordext + 1, NT * 128).astype(np.int64)].astype(np.float32).reshape(NB, 128).T.copy()
        per_core.append(dict(idxw=idxw, dloc=dlocw, rdA=rdA, rdB=rdB,
                             rsA=lo, rsB=hi, order=orders[c]))

    meta = dict(
        cfg=cfg, NPC=NPC, NB=NB, NPAD=NPAD, NT=NT, NBS=NBS, E=E,
        orders=orders,
        K=K, nA=nA, nB=nB_, NCHUNK=NCHUNK,
        calls=calls, mm=mm, PG=PG, pgroups=pgroups,
        last_mm_of_block=last_mm_of_block,
        last_mm_of_call=last_mm_of_call,
        first_mm_of_call=first_mm_of_call,
        first_mm_of_pg=first_mm_of_pg, last_mm_of_pg=last_mm_of_pg,
    )
    return meta, per_core


# --------------------------------------------------------------------------
# Device program
# --------------------------------------------------------------------------

def build_nc(meta, skip_phase1=False, skip_phase2=False, reps=1, gathers_only=False):
    cfg = meta["cfg"]
    D = cfg["D"]
    SPLIT, GCH, HBT = cfg["SPLIT"], cfg["GCH"], cfg["HBT"]
    NSLOT = cfg["NSLOT"]
    NB, NPAD, NT, NCHUNK = meta["NB"], meta["NPAD"], meta["NT"], meta["NCHUNK"]
    NPC = meta["NPC"]
    nA = meta["nA"]
    calls, mm = meta["calls"], meta["mm"]
    last_of_block = meta["last_mm_of_block"]
    last_of_call = meta["last_mm_of_call"]
    first_of_call = meta["first_mm_of_call"]
    PG, pgroups = meta["PG"], meta["pgroups"]
    first_mm_of_pg, last_mm_of_pg = meta["first_mm_of_pg"], meta["last_mm_of_pg"]
    CORES = cfg["CORES"]
    NROWS = NT * 128
    NPSUM = cfg.get("NPSUM", 4)

    ncalls_q = {q: sum(1 for c in calls if c[0] == q) for q in (0, 1)}
    npg_q = {q: sum(1 for g in pgroups if g[0] == q) for q in (0, 1)}

    nc = bacc.Bacc(num_swdge_queues=4, dynamic_dma_scratch_size=4 * 16384)

    hepi_p = nc.declare_dram_parameter("hepi", [NPAD, D], F32, isOutput=False)
    rsA_p = nc.declare_dram_parameter("rsa", [128, NB], F32, isOutput=False)
    rsB_p = nc.declare_dram_parameter("rsb", [128, NB], F32, isOutput=False)
    rdA_p = nc.declare_dram_parameter("rda", [128, NB], F32, isOutput=False)
    rdB_p = nc.declare_dram_parameter("rdb", [128, NB], F32, isOutput=False)
    dloc_p = nc.declare_dram_parameter("dloc", [128, NCHUNK], F32, isOutput=False)
    idxw_p = nc.declare_dram_parameter("idxw", [128, NCHUNK * 8], I16, isOutput=False)
    ks_p = nc.declare_dram_parameter("ks", [128, D], F32, isOutput=False)
    iot_p = nc.declare_dram_parameter("iot", [128, 128], F32, isOutput=False)
    out_p = nc.declare_dram_parameter("out", [NPAD, D], F32, isOutput=True)

    hk = nc.dram_tensor("hk", [NROWS, D], BF16)          # all-gathered
    hks = nc.dram_tensor("hks", [NPAD, D], BF16)         # own slice

    hks_t = hks[:, :].rearrange("(t p) f -> p t f", p=128)
    hepi_t = hepi_p[:, :].rearrange("(b p) f -> p b f", p=128)  # [128, NB, D]

    groups = [(g0, min(HBT, NB - g0)) for g0 in range(0, NB, HBT)]
    NG = len(groups)
    tthru = np.concatenate([[0], np.cumsum([t for _, t in groups])]).astype(int)
    # per-rep semaphore increment totals
    cntg = [sum(1 for x in range(NG) if x % 2 == i) for i in range(2)]
    T_hst = [16 * cntg[i] for i in range(2)]
    HALF = NPC // 2
    RH = CORES * HALF                     # region size in hk rows
    # group index after which each half of hks is fully stored
    gdone0 = next(gi for gi, (g0, tg) in enumerate(groups)
                  if (g0 + tg) * 128 >= HALF)
    # per-parity store counts through group g (inclusive)
    def _st_thru(gi):
        return [sum(1 for x in range(gi + 1) if x % 2 == i) for i in range(2)]
    T_out = [16 * sum(1 for b in range(NB) if b % 2 == i) for i in range(2)]
    T_gq = {(q, s): 16 * sum(1 for c in calls if c[0] == q and c[3] % NSLOT == s)
            for q in (0, 1) for s in range(NSLOT)}

    with ExitStack() as ctx:
        rsa = ctx.enter_context(nc.sbuf_tensor("b_rsa", [128, NB], F32))
        rsb = ctx.enter_context(nc.sbuf_tensor("b_rsb", [128, NB], F32))
        rda = ctx.enter_context(nc.sbuf_tensor("b_rda", [128, NB], F32))
        rdb = ctx.enter_context(nc.sbuf_tensor("b_rdb", [128, NB], F32))
        dloc = ctx.enter_context(nc.sbuf_tensor("b_dloc", [128, NCHUNK], F32))
        idxw = ctx.enter_context(nc.sbuf_tensor("b_idxw", [128, NCHUNK * 8], I16))
        ks = ctx.enter_context(nc.sbuf_tensor("b_ks", [128, D], F32))
        kn = ctx.enter_context(nc.sbuf_tensor("b_kn", [128, D], F32))
        iota_f = ctx.enter_context(nc.sbuf_tensor("b_iota", [128, 128], F32))
        hepi = ctx.enter_context(nc.sbuf_tensor("b_hepi", [128, NB, D], F32))
        hepik = ctx.enter_context(nc.sbuf_tensor("b_hepik", [128, NB, D], F32))
        hbout = ctx.enter_context(nc.sbuf_tensor("hbout", [128, 2 * HBT, D], BF16))
        gbufA = ctx.enter_context(nc.sbuf_tensor("gbufA", [128, NSLOT * GCH, D], BF16))
        gbufB = ctx.enter_context(nc.sbuf_tensor("gbufB", [128, NSLOT * GCH, D], BF16))
        pbufA = ctx.enter_context(nc.sbuf_tensor("pbufA", [128, 2 * PG, 128], BF16))
        pbufB = ctx.enter_context(nc.sbuf_tensor("pbufB", [128, 2 * PG, 128], BF16))
        outb = ctx.enter_context(nc.sbuf_tensor("outb", [128, 2, D], F32))
        psums = [ctx.enter_context(nc.psum_tensor(f"ps{i}", [128, D], F32))
                 for i in range(NPSUM)]
        s_ld = ctx.enter_context(nc.semaphore("s_ld"))
        s_v = ctx.enter_context(nc.semaphore("s_v"))
        s_pre = ctx.enter_context(nc.semaphore("s_pre"))
        s_hsc = ctx.enter_context(nc.semaphore("s_hsc"))
        s_hst = [ctx.enter_context(nc.semaphore(f"s_hst{i}")) for i in range(2)]
        s_cc = ctx.enter_context(nc.semaphore("s_cc"))
        s_g = {(q, i): ctx.enter_context(nc.semaphore(f"s_g{q}{i}"))
               for q in (0, 1) for i in range(NSLOT)}
        s_pq = [ctx.enter_context(nc.semaphore(f"s_p{q}")) for q in (0, 1)]
        s_mm = ctx.enter_context(nc.semaphore("s_mm"))
        s_epi = ctx.enter_context(nc.semaphore("s_epi"))
        s_out = [ctx.enter_context(nc.semaphore(f"s_out{i}")) for i in range(2)]

        pool_prog = []
        for (q, s0, nch, k) in calls:
            need = 0
            if k >= NSLOT:
                need = last_of_call[(q, k - NSLOT)] + 1
            # issue initial q0 calls before blocking on the second collective
            pool_prog.append(((need, q, first_of_call[(q, k)]), ("call", q, s0, nch, k)))
        pool_prog.sort()

        with nc.Block("all") as blk:

            @blk.gpsimd
            def _(g):
                g.dma_start(out=rsa[:, :], in_=rsA_p[:, :]).then_inc(s_ld, 16)
                g.dma_start(out=rsb[:, :], in_=rsB_p[:, :]).then_inc(s_ld, 16)
                g.dma_start(out=rda[:, :], in_=rdA_p[:, :]).then_inc(s_ld, 16)
                g.dma_start(out=rdb[:, :], in_=rdB_p[:, :]).then_inc(s_ld, 16)
                g.dma_start(out=dloc[:, :], in_=dloc_p[:, :]).then_inc(s_ld, 16)
                g.dma_start(out=idxw[:, :], in_=idxw_p[:, :]).then_inc(s_ld, 16)
                g.dma_start(out=ks[:, :], in_=ks_p[:, :]).then_inc(s_ld, 16)
                g.dma_start(out=iota_f[:, :], in_=iot_p[:, :]).then_inc(s_ld, 16)
                g.dma_start(out=hepi[:, :, :], in_=hepi_t).then_inc(s_ld, 16)
                for r in range(reps):
                    if not skip_phase1:
                        # hk slice stores, split all-gather (half overlaps gathers)
                        for gi, (g0, tg) in enumerate(groups):
                            if gi == 0 and r > 0:
                                g.wait_ge(s_cc, 2 * r)  # collectives r-1 read hks
                            g.wait_ge(s_hsc, r * NB + int(tthru[gi + 1]))
                            g.dma_start(
                                out=hks_t[:, g0:g0 + tg, :],
                                in_=hbout[:, (gi % 2) * HBT:(gi % 2) * HBT + tg, :]
                                ).then_inc(s_hst[gi % 2], 16)
                            if gi == gdone0:
                                st = _st_thru(gi)
                                for i in range(2):
                                    g.wait_ge(s_hst[i], 16 * (r * cntg[i] + st[i]))
                                if r > 0 and not skip_phase2:
                                    g.wait_ge(s_mm, r * NCHUNK)
                                g.collective_compute(
                                    "AllGather",
                                    mybir.AluOpType.bypass,
                                    replica_groups=[list(range(CORES))],
                                    ins=[hks[0:HALF, :]],
                                    outs=[hk[0:RH, :]],
                                ).then_inc(s_cc, 1)
                        for i in range(2):
                            g.wait_ge(s_hst[i], (r + 1) * T_hst[i])
                        if r > 0 and not skip_phase2:
                            g.wait_ge(s_mm, r * NCHUNK)
                        g.collective_compute(
                            "AllGather",
                            mybir.AluOpType.bypass,
                            replica_groups=[list(range(CORES))],
                            ins=[hks[HALF:2 * HALF, :]],
                            outs=[hk[RH:2 * RH, :]],
                        ).then_inc(s_cc, 1)
                    if skip_phase2:
                        continue
                    if not skip_phase1:
                        g.wait_ge(s_cc, 2 * r + 1)
                        cc1_waited = False
                    cur = -1
                    for _, item in pool_prog:
                        _, q, s0, nch, k = item
                        kg = r * ncalls_q[q] + k
                        if gathers_only:
                            need = 0
                        elif kg >= NSLOT:
                            rr, kk = divmod(kg - NSLOT, ncalls_q[q])
                            need = rr * NCHUNK + last_of_call[(q, kk)] + 1
                        else:
                            need = 0
                        if need > cur:
                            g.wait_ge(s_mm, need)
                            cur = need
                        if q == 1 and not skip_phase1 and not cc1_waited:
                            g.wait_ge(s_cc, 2 * r + 2)
                            cc1_waited = True
                        gb = gbufA if q == 0 else gbufB
                        base_col = (0 if q == 0 else nA) * 8
                        src_view = hk[0:RH, :] if q == 0 else hk[RH:NROWS, :]
                        slot = (k % NSLOT) * GCH
                        g.dma_gather(
                            out_ap=gb[:, slot:slot + nch, :],
                            in_ap=src_view,
                            idxs_ap=idxw[:, base_col + s0 * 8: base_col + (s0 + nch) * 8],
                            num_idxs=nch * 128,
                            num_idxs_reg=nch * 128,
                            elem_size=D,
                            queue_num=(q * 2 + k) % 4,
                        ).then_inc(s_g[(q, k % NSLOT)], 16)

            @blk.vector
            def _(v):
                v.wait_ge(s_ld, 144)
                entries = ([("pg", g, first_mm_of_pg[(g[0], g[1])], 0) for g in pgroups]
                           + [("epi", b, last_of_block[b], 1) for b in range(NB)])
                entries.sort(key=lambda x: (x[2], x[3]))
                pbufs = [pbufA, pbufB]
                for r in range(reps):
                    sv0, sp0 = r * 8, r * 2
                    # kn = 1 - K
                    v.memset(kn[:, :], 1.0).then_inc(s_v, 1)
                    v.wait_ge(s_v, sv0 + 1)
                    v.tensor_tensor(out=kn[:, :], in0=kn[:, :], in1=ks[:, :],
                                    op=OP.subtract).then_inc(s_v, 1)
                    v.tensor_tensor(out=rsa[:, :], in0=rsb[:, :], in1=rsa[:, :],
                                    op=OP.subtract).then_inc(s_v, 1)
                    v.wait_ge(s_v, sv0 + 3)
                    v.tensor_scalar(out=rsa[:, :], in0=rsa[:, :], scalar1=1.0,
                                    scalar2=None, op0=OP.max).then_inc(s_v, 1)
                    v.wait_ge(s_v, sv0 + 4)
                    v.reciprocal(rsa[:, :], rsa[:, :]).then_inc(s_v, 1)
                    v.tensor_tensor(out=rda[:, :], in0=rdb[:, :], in1=rda[:, :],
                                    op=OP.subtract).then_inc(s_v, 1)
                    v.wait_ge(s_v, sv0 + 6)
                    v.tensor_scalar(out=rda[:, :], in0=rda[:, :], scalar1=1.0,
                                    scalar2=None, op0=OP.max).then_inc(s_v, 1)
                    v.wait_ge(s_v, sv0 + 7)
                    v.reciprocal(rda[:, :], rda[:, :]).then_inc(s_v, 1)
                    if not skip_phase1:
                        v.wait_ge(s_pre, sp0 + 1)
                        for gi, (g0, tg) in enumerate(groups):
                            if r > 0 or gi >= 2:
                                v.wait_ge(s_hst[gi % 2],
                                          16 * (r * cntg[gi % 2] + gi // 2))
                            for i in range(tg):
                                t = g0 + i
                                v.scalar_tensor_tensor(
                                    out=hbout[:, (gi % 2) * HBT + i, :],
                                    in0=hepi[:, t, :],
                                    scalar=rsa[:, t:t + 1], in1=ks[:, :],
                                    op0=OP.mult, op1=OP.mult).then_inc(s_hsc, 1)
                    v.wait_ge(s_v, sv0 + 2)
                    for b in range(NB):
                        v.tensor_tensor(out=hepik[:, b, :], in0=hepi[:, b, :],
                                        in1=kn[:, :], op=OP.mult)
                    if skip_phase2 or gathers_only:
                        continue
                    v.wait_ge(s_pre, sp0 + 2)  # rda sqrt done (epilogue reads rda)
                    for kind, item, _, _ in entries:
                        if kind == "pg":
                            q, j, posq0, n = item
                            jg = r * npg_q[q] + j
                            if jg >= 2:
                                rr, jj = divmod(jg - 2, npg_q[q])
                                v.wait_ge(s_mm, rr * NCHUNK + last_mm_of_pg[(q, jj)] + 1)
                            sid0 = posq0 + (0 if q == 0 else nA)
                            slot0 = (j % 2) * PG
                            v.tensor_tensor(
                                out=pbufs[q][:, slot0:slot0 + n, :],
                                in0=bass.AP(iota_f, 0, [[128, 128], [0, n], [1, 128]]),
                                in1=bass.AP(dloc, sid0, [[NCHUNK, 128], [1, n], [0, 128]]),
                                op=OP.is_equal).then_inc(s_pq[q], 1)
                        else:
                            b = item
                            bg = r * NB + b
                            cnt = sum(1 for x in range(NB) if x % 2 == b % 2)
                            v.wait_ge(s_mm, r * NCHUNK + last_of_block[b] + 1)
                            if bg >= 2:
                                v.wait_ge(s_out[b % 2], 16 * (r * cnt + b // 2))
                            v.scalar_tensor_tensor(
                                out=outb[:, b % 2, :], in0=psums[b % NPSUM][:, :],
                                scalar=rda[:, b:b + 1], in1=hepik[:, b, :],
                                op0=OP.mult, op1=OP.add).then_inc(s_epi, 1)

            @blk.scalar
            def _(s):
                for r in range(reps):
                    s.wait_ge(s_v, r * 8 + 5)
                    s.sqrt(rsa[:, :], rsa[:, :]).then_inc(s_pre, 1)
                    s.wait_ge(s_v, r * 8 + 8)
                    s.sqrt(rda[:, :], rda[:, :]).then_inc(s_pre, 1)

            @blk.tensor
            def _(t):
                pbufs = [pbufA, pbufB]
                for r in range(reps):
                    if skip_phase2 or gathers_only:
                        continue
                    pg_waited = {0: -1, 1: -1}
                    for m, e in enumerate(mm):
                        q, k, b = e["q"], e["callq"], e["b"]
                        bg = r * NB + b
                        if first_of_call[(q, k)] == m:
                            t.wait_ge(s_g[(q, k % NSLOT)],
                                      r * T_gq[(q, k % NSLOT)] + 16 * (k // NSLOT + 1))
                        if e["first"] and bg >= NPSUM:
                            t.wait_ge(s_epi, bg - NPSUM + 1)
                        j = e["posq"] // PG
                        if j > pg_waited[q]:
                            t.wait_ge(s_pq[q], r * npg_q[q] + j + 1)
                            pg_waited[q] = j
                        gb = gbufA if q == 0 else gbufB
                        slot = (k % NSLOT) * GCH + (e["posq"] - k * GCH)
                        t.matmul(
                            out=psums[b % NPSUM][:, :],
                            lhsT=pbufs[q][:, e["posq"] % (2 * PG), :],
                            rhs=gb[:, slot, :],
                            start=e["first"], stop=e["last"],
                        ).then_inc(s_mm, 1)

            @blk.sync
            def _(sp):
                for r in range(reps):
                    if skip_phase2 or gathers_only:
                        continue
                    for b in range(NB):
                        sp.wait_ge(s_epi, r * NB + b + 1)
                        sp.dma_start(out=out_p[b * 128:(b + 1) * 128, :],
                                     in_=outb[:, b % 2, :]).then_inc(s_out[b % 2], 16)

    nc.compile()
    return nc


# --------------------------------------------------------------------------
# Full pipeline: numpy in -> numpy out
# --------------------------------------------------------------------------

def prepare(H, K_channel_wise, src, dst, cfg=None):
    meta, per_core = make_plan(src, dst, cfg)
    cfg = meta["cfg"]
    N, D, CORES = cfg["N"], cfg["D"], cfg["CORES"]
    NT, NPC, NPAD = meta["NT"], meta["NPC"], meta["NPAD"]
    H = np.asarray(H, dtype=np.float32)
    Ks = np.broadcast_to(np.asarray(K_channel_wise, np.float32).reshape(1, D),
                         (128, D)).copy()
    iot = np.broadcast_to(np.arange(128), (128, 128)).astype(np.float32)
    in_maps = []
    for c in range(CORES):
        pc = per_core[c]
        hepi = np.zeros((NPAD, D), dtype=np.float32)
        hepi[:NPC] = H[c * NPC:(c + 1) * NPC][pc["order"]]
        in_maps.append(dict(
            hepi=hepi, rsa=pc["rsA"], rsb=pc["rsB"],
            rda=pc["rdA"], rdb=pc["rdB"], dloc=pc["dloc"], idxw=pc["idxw"],
            ks=Ks, iot=iot,
        ))
    return meta, in_maps


def gather_output(meta, results):
    cfg = meta["cfg"]
    N, CORES, NPC = cfg["N"], cfg["CORES"], meta["NPC"]
    full = np.empty((N, cfg["D"]), dtype=np.float32)
    for c in range(CORES):
        full[c * NPC + meta["orders"][c]] = results[c]["out"][:NPC]
    return full


def kernel(H, K_channel_wise, src, dst):
    """Entry point: FULL (unsharded) inputs -> FULL [50000, 128] f32 output.

    Shards edges by dst across the 8 NeuronCores (with a per-core
    block-balancing node permutation to minimize chunk padding), compiles
    and runs the SPMD Bass kernel, and un-permutes the per-core outputs.
    """
    meta, in_maps = prepare(H, K_channel_wise, src, dst)
    nc = build_nc(meta)
    res = run_bass_kernel_spmd(nc, in_maps,
                               core_ids=list(range(meta["cfg"]["CORES"])))
    return gather_output(meta, res.results).astype(np.float32)

